# revision 1
# baseline (speedup 1.0000x reference)
"""Trainium2 Bass kernel for nn_CoXtLayer (CoTNeXt-style layer).

Sharding: pure data-parallel over batch — 16 images over 8 cores = 2 images
per core; all parameters host-folded and replicated.

Per-core pipeline (channels-on-partitions, 58x58 zero-padded spatial tiles):
  ke   : grouped 3x3 conv as 9 PSUM-accumulated block-diag matmuls, ACT
         Relu+bias drain with fused GAP accumulation
  em1  : 1x1 conv on interleaved qk via host de-interleaved weights (reads
         x and k directly, qk never materialized)
  c1   : 1x1 conv -> v (both c1-BN and y-BN scales folded into weights)
  em2  : pass 1 compact (288ch) matmul + bn_stats -> GroupNorm stats via
         indicator matmuls; pass 2 emits the per-pixel dynamic kernels
         directly SP-expanded on 128 partitions, with GN scale (rho*gamma)
         folded into runtime-scaled lhsT and the bias via a K=65 ones-row
  local: 9 DVE tensor_tensor multiplies against shifted v views + bf16
         pairwise tree sum; ACT Relu drain -> y (+ fused GAP)
  SE   : radix-2 softmax == sigmoid of host pre-subtracted weight diff
"""
import numpy as np
import ml_dtypes
from contextlib import ExitStack

import concourse.bass as bass
import concourse.mybir as mybir
import concourse.tile as tile
from concourse.bass_utils import run_bass_kernel_spmd

F32 = mybir.dt.float32
BF16 = mybir.dt.bfloat16
AF = mybir.ActivationFunctionType
ALU = mybir.AluOpType
BF = ml_dtypes.bfloat16

DIM = 256
KK = 3
B = 16
H = 56
W = 56
DWG = 2
SP = 8
RADIX = 2
ATTN = 128
EPS = 1e-5
NCORES = 8
BPC = B // NCORES          # 2 images per core
HP = H + 2
WP = W + 2
NPIX = H * W               # 3136
CH = 8                     # chunk rows
NCH = H // CH              # 7 chunks
CHUNK = CH * W             # 448
WC = 288                   # dynamic-kernel channels
TAPS = [(kh, kw) for kh in range(3) for kw in range(3)]
FUSED_ROWS = set()         # kh-rows whose 3 taps multiply straight from PSUM
                           # on DVE; other rows drain wide on ACT then
                           # multiply wide on DVE

_trace_flag = [False]
_last_result = [None]


# ---------------------------------------------------------------------------
# wait splitting: this walrus rejects instructions with >1 sync-wait command
# ---------------------------------------------------------------------------
_nop_ctr = [0]


def _split_waits(nc, max_waits=1):
    for fn in nc.m.functions:
        for blk in fn.blocks:
            out = []
            for inst in blk.instructions:
                si = inst.sync_info
                waits = list(si.on_wait) if (si and si.on_wait) else []
                if len(waits) > max_waits:
                    for wcond in waits[max_waits:]:
                        _nop_ctr[0] += 1
                        out.append(mybir.InstNoOp(
                            name=f"waitnop-{_nop_ctr[0]}",
                            engine=inst.engine, ins=[], outs=[],
                            sync_info=mybir.SyncInfo(on_wait=[wcond],
                                                     on_update=[]),
                        ))
                    si.on_wait = waits[:max_waits]
                    inst.sync_info = si
                out.append(inst)
            try:
                blk.instructions = out
            except Exception:
                while len(blk.instructions):
                    blk.instructions.pop()
                for i in out:
                    blk.instructions.append(i)
    return nc


# ---------------------------------------------------------------------------
# host-side weight folding
# ---------------------------------------------------------------------------
def _fold(inputs):
    f32 = np.float32
    s = lambda g: (g / np.sqrt(f32(1.0 + EPS))).astype(f32)
    d = {}
    # --- ke: grouped 3x3, groups=8, block-diag lhsT [oct][K=ic, tap, M=oc]
    ke_w = inputs["ke_w"].astype(f32)
    s_ke = s(inputs["ke_g"])
    kew = np.zeros((2, 128, 9, 128), f32)
    for oct_ in range(2):
        for gl in range(4):
            oc0 = oct_ * 128 + gl * 32
            for t, (kh, kw) in enumerate(TAPS):
                blk = ke_w[oc0:oc0 + 32, :, kh, kw] * s_ke[oc0:oc0 + 32, None]
                kew[oct_, gl * 32:gl * 32 + 32, t, gl * 32:gl * 32 + 32] = blk.T
    d["kew"] = kew.astype(BF)
    d["b_ke"] = inputs["ke_b"].astype(f32).reshape(2, 128)
    # --- em1: de-interleaved qk weights; per dwg: [src(x=0,k=1)][K=128, M=64]
    em1_w = inputs["em1_w"][:, :, 0, 0].astype(f32)
    s1 = s(inputs["em1_g"])
    em1 = np.zeros((2, 2, 128, 64), f32)
    for dwg in range(2):
        for ocl in range(64):
            oc = dwg * 64 + ocl
            em1[dwg, 0, :, ocl] = em1_w[oc, 0::2] * s1[oc]
            em1[dwg, 1, :, ocl] = em1_w[oc, 1::2] * s1[oc]
    d["em1w"] = em1.astype(BF)
    d["b_em1"] = (inputs["em1_b"].astype(f32) * 1.0).reshape(2, 64)
    # --- c1: fold s_c1*s_y into weights, b = c1_b*s_y; lhsT [dwg][K=ic, M=oc]
    c1_w = inputs["c1_w"][:, :, 0, 0].astype(f32)
    s_c1 = s(inputs["c1_g"])
    s_y = s(inputs["bn_g"])
    wv = c1_w * (s_c1 * s_y)[:, None]
    c1 = np.stack([wv[0:128].T, wv[128:256].T], 0)
    d["c1w"] = c1.astype(BF)
    d["b_v"] = (inputs["c1_b"].astype(f32) * s_y).reshape(2, 128)
    d["b_y"] = inputs["bn_b"].astype(f32).reshape(2, 128)
    # --- em2 pass1 compact lhsT + per-wc bias + group indicators
    em2_w = inputs["em2_w"][:, :, 0, 0].astype(f32)     # [288, 64]
    em2_b = inputs["em2_bias"].astype(f32)
    d["em2c0"] = np.stack([em2_w[0:128].T, em2_w[144:272].T], 0).astype(BF)
    d["em2c1"] = np.stack([em2_w[128:144].T, em2_w[272:288].T], 0).astype(BF)
    d["bem2v0"] = np.stack([em2_b[0:128], em2_b[144:272]], 0)
    d["bem2v1"] = np.stack([em2_b[128:144], em2_b[272:288]], 0)
    ind0 = np.zeros((2, 128, 32), f32)
    ind1 = np.zeros((2, 16, 32), f32)
    for dwg in range(2):
        for l in range(128):
            ind0[dwg, l, dwg * 16 + l // 9] = 1.0
        for l in range(16):
            ind1[dwg, l, dwg * 16 + (128 + l) // 9] = 1.0
    d["ind0"] = ind0
    d["ind1"] = ind1
    # --- em2 pass2 expanded static weights (gn_g folded) + [9,128] T-layout
    # vectors for the runtime beta row: beta = (be2 - mu)*gn_g*rho + gn_b
    gn_g = inputs["gn_g"].astype(f32)
    gn_b = inputs["gn_b"].astype(f32)
    w2x = np.zeros((2, 64, 9, 128), f32)
    gngT = np.zeros((2, 9, 128), f32)
    gnbT = np.zeros((2, 9, 128), f32)
    begT = np.zeros((2, 9, 128), f32)   # be2 * gn_g pre-folded
    for dwg in range(2):
        for t in range(9):
            for cp in range(128):
                wc = dwg * 144 + (cp // 8) * 9 + t
                w2x[dwg, :, t, cp] = em2_w[wc] * gn_g[wc]
                gngT[dwg, t, cp] = gn_g[wc]
                gnbT[dwg, t, cp] = gn_b[wc]
                begT[dwg, t, cp] = em2_b[wc] * gn_g[wc]
    d["w2x"] = w2x.astype(BF)
    d["gngT"] = gngT
    d["gnbT"] = gnbT
    d["begT"] = begT
    # --- SE
    se1_w = inputs["se1_w"][:, :, 0, 0].astype(f32)     # [128, 256]
    s_se = s(inputs["se_g"])
    se1f = se1_w * s_se[:, None]
    d["se1w"] = np.stack([se1f[:, 0:128].T, se1f[:, 128:256].T], 0)
    d["b_se1"] = inputs["se1_bias"].astype(f32) * s_se + inputs["se_b"].astype(f32)
    se2_w = inputs["se2_w"][:, :, 0, 0].astype(f32)     # [512, 128]
    se2_b = inputs["se2_bias"].astype(f32)
    dw = se2_w[0::2] - se2_w[1::2]                      # [256, 128]
    db = se2_b[0::2] - se2_b[1::2]
    d["se2dw"] = np.stack([dw[0:128].T, dw[128:256].T], 0)
    d["d_b"] = db.reshape(2, 128)
    d["ident32"] = np.eye(32, dtype=f32)
    d["ones_row"] = np.ones((1, H * W), BF)
    return d


# ---------------------------------------------------------------------------
# device kernel
# ---------------------------------------------------------------------------
def _build():
    nc = bass.Bass()
    dt = nc.dram_tensor
    x_d = dt("x_in", [BPC, DIM, H, W], BF16, kind="ExternalInput")
    kew_d = dt("kew", [2, 128, 9, 128], BF16, kind="ExternalInput")
    bke_d = dt("b_ke", [2, 128], F32, kind="ExternalInput")
    em1w_d = dt("em1w", [2, 2, 128, 64], BF16, kind="ExternalInput")
    bem1_d = dt("b_em1", [2, 64], F32, kind="ExternalInput")
    c1w_d = dt("c1w", [2, 128, 128], BF16, kind="ExternalInput")
    bv_d = dt("b_v", [2, 128], F32, kind="ExternalInput")
    by_d = dt("b_y", [2, 128], F32, kind="ExternalInput")
    em2c0_d = dt("em2c0", [2, 64, 128], BF16, kind="ExternalInput")
    em2c1_d = dt("em2c1", [2, 64, 16], BF16, kind="ExternalInput")
    bem2v0_d = dt("bem2v0", [2, 128], F32, kind="ExternalInput")
    bem2v1_d = dt("bem2v1", [2, 16], F32, kind="ExternalInput")
    ind0_d = dt("ind0", [2, 128, 32], F32, kind="ExternalInput")
    ind1_d = dt("ind1", [2, 16, 32], F32, kind="ExternalInput")
    w2x_d = dt("w2x", [2, 64, 9, 128], BF16, kind="ExternalInput")
    gngT_d = dt("gngT", [2, 9, 128], F32, kind="ExternalInput")
    gnbT_d = dt("gnbT", [2, 9, 128], F32, kind="ExternalInput")
    begT_d = dt("begT", [2, 9, 128], F32, kind="ExternalInput")
    se1w_d = dt("se1w", [2, 128, 128], F32, kind="ExternalInput")
    bse1_d = dt("b_se1", [128], F32, kind="ExternalInput")
    se2dw_d = dt("se2dw", [2, 128, 128], F32, kind="ExternalInput")
    db_d = dt("d_b", [2, 128], F32, kind="ExternalInput")
    id32_d = dt("ident32", [32, 32], F32, kind="ExternalInput")
    ones_d = dt("ones_row", [1, NPIX], BF16, kind="ExternalInput")
    out_d = dt("out", [BPC, DIM, H, W], BF16, kind="ExternalOutput")
    scr_d = dt("scr", [BPC, 2, 32], F32, kind="Internal")
    scr2_d = dt("scr2", [BPC, 2, 2, 128], BF16, kind="Internal")

    with tile.TileContext(nc) as tc, ExitStack() as ctx:
        const = ctx.enter_context(tc.tile_pool(name="const", bufs=1))
        xpool = ctx.enter_context(tc.tile_pool(name="xpad", bufs=3))
        xspool = ctx.enter_context(tc.tile_pool(name="xstage", bufs=1))
        kpool = ctx.enter_context(tc.tile_pool(name="ktile", bufs=4))
        upool = ctx.enter_context(tc.tile_pool(name="uex", bufs=4))
        vpool = ctx.enter_context(tc.tile_pool(name="vpad", bufs=4))
        wpool = ctx.enter_context(tc.tile_pool(name="wchunk", bufs=2))
        zpool = ctx.enter_context(tc.tile_pool(name="zchunk", bufs=2))
        ypool = ctx.enter_context(tc.tile_pool(name="ytile", bufs=4))
        lpool = ctx.enter_context(tc.tile_pool(name="lhsT", bufs=4))
        spool = ctx.enter_context(tc.tile_pool(name="stats", bufs=4))
        mpool = ctx.enter_context(tc.tile_pool(name="mix", bufs=2))
        psD = ctx.enter_context(tc.tile_pool(name="psD", bufs=1, space="PSUM"))
        psE = ctx.enter_context(tc.tile_pool(name="psE", bufs=3, space="PSUM"))
        psA = ctx.enter_context(tc.tile_pool(name="psA", bufs=1, space="PSUM"))
        psB = ctx.enter_context(tc.tile_pool(name="psB", bufs=1, space="PSUM"))
        psC = ctx.enter_context(tc.tile_pool(name="psC", bufs=1, space="PSUM"))
        psM = ctx.enter_context(tc.tile_pool(name="psM", bufs=1, space="PSUM"))

        # ---- static weights to SBUF ----
        def ld(dram_ap, shape, dtype, name):
            t = const.tile(shape, dtype, tag=name)
            nc.sync.dma_start(out=t, in_=dram_ap)
            return t

        kew = [ld(kew_d[o], [128, 9, 128], BF16, f"kew{o}") for o in range(2)]
        bke = [ld(bke_d[o][:, None], [128, 1], F32, f"bke{o}") for o in range(2)]
        em1w = [[ld(em1w_d[g, sderiv], [128, 64], BF16, f"em1w{g}{sderiv}")
                 for sderiv in range(2)] for g in range(2)]
        bem1 = [ld(bem1_d[g][:, None], [64, 1], F32, f"bem1{g}") for g in range(2)]
        c1w = [ld(c1w_d[g], [128, 128], BF16, f"c1w{g}") for g in range(2)]
        bv = [ld(bv_d[g][:, None], [128, 1], F32, f"bv{g}") for g in range(2)]
        by = [ld(by_d[g][:, None], [128, 1], F32, f"by{g}") for g in range(2)]
        em2c = [[ld(em2c0_d[g], [64, 128], BF16, f"em2c0{g}"),
                 ld(em2c1_d[g], [64, 16], BF16, f"em2c1{g}")] for g in range(2)]
        bem2v = [[ld(bem2v0_d[g][:, None], [128, 1], F32, f"bem2v0{g}"),
                  ld(bem2v1_d[g][:, None], [16, 1], F32, f"bem2v1{g}")]
                 for g in range(2)]
        ind = [[ld(ind0_d[g], [128, 32], F32, f"ind0{g}"),
                ld(ind1_d[g], [16, 32], F32, f"ind1{g}")] for g in range(2)]
        w2x = [ld(w2x_d[g], [64, 9, 128], BF16, f"w2x{g}") for g in range(2)]
        gngT = [ld(gngT_d[g], [9, 128], F32, f"gngT{g}") for g in range(2)]
        gnbT = [ld(gnbT_d[g], [9, 128], F32, f"gnbT{g}") for g in range(2)]
        begT = [ld(begT_d[g], [9, 128], F32, f"begT{g}") for g in range(2)]
        se1w = [ld(se1w_d[i], [128, 128], F32, f"se1w{i}") for i in range(2)]
        bse1 = ld(bse1_d[:][:, None], [128, 1], F32, "bse1")
        se2dw = [ld(se2dw_d[i], [128, 128], F32, f"se2dw{i}") for i in range(2)]
        db_t = [ld(db_d[i][:, None], [128, 1], F32, f"db{i}") for i in range(2)]
        id32 = ld(id32_d[:, :], [32, 32], F32, "id32")
        epsv = const.tile([128, 1], F32, tag="epsv")
        nc.vector.memset(epsv, EPS)

        # per-image state
        xp = [[None, None] for _ in range(BPC)]
        kt = [[None, None] for _ in range(BPC)]
        uex = [[None, None] for _ in range(BPC)]
        vp = [[None, None] for _ in range(BPC)]
        yt = [[None, None] for _ in range(BPC)]
        gk = [[None, None] for _ in range(BPC)]
        gy = [[None, None] for _ in range(BPC)]
        lall = [[None, None] for _ in range(BPC)]
        betas = [[None, None] for _ in range(BPC)]

        def pad_guards(t):
            nc.gpsimd.memset(t[:, 0, :], 0.0)
            nc.gpsimd.memset(t[:, HP - 1, :], 0.0)
            nc.gpsimd.memset(t[:, 1:HP - 1, 0:1], 0.0)
            nc.gpsimd.memset(t[:, 1:HP - 1, WP - 1:WP], 0.0)

        def stage_ABCDE(img):
            # A: load x
            for o in range(2):
                xs = xspool.tile([128, NPIX], BF16, tag="xs")
                nc.sync.dma_start(out=xs, in_=x_d[img, o * 128:(o + 1) * 128])
                t = xpool.tile([128, HP, WP], BF16, tag="xp")
                pad_guards(t)
                nc.vector.tensor_copy(
                    out=t[:, 1:1 + H, 1:1 + W],
                    in_=xs.rearrange("p (a b) -> p a b", a=H))
                xp[img][o] = t
            # B: ke conv -> k
            for o in range(2):
                ktile = kpool.tile([128, H, W], BF16, tag="k")
                gkt = spool.tile([128, NCH], F32, tag="gapk")
                for c in range(NCH):
                    h0 = c * CH
                    ps = psA.tile([128, CH, W], F32, tag="ke")
                    for j, (kh, kw) in enumerate(TAPS):
                        nc.tensor.matmul(
                            ps, kew[o][:, j, :],
                            xp[img][o][:, h0 + kh:h0 + kh + CH, kw:kw + W],
                            start=(j == 0), stop=(j == 8))
                    nc.scalar.activation(
                        out=ktile[:, h0:h0 + CH, :], in_=ps, func=AF.Relu,
                        bias=bke[o], scale=1.0, accum_out=gkt[:, c:c + 1])
                kt[img][o] = ktile
                gk[img][o] = gkt
            # C: em1 -> uex (row 64 = ones so the lhsT beta row adds a bias)
            for g in range(2):
                u = upool.tile([65, NPIX], BF16, tag="uex")
                nc.sync.dma_start(out=u[64:65, :], in_=ones_d[:, :])
                uex[img][g] = u
            for c in range(NCH):
                h0 = c * CH
                csl = slice(c * CHUNK, (c + 1) * CHUNK)
                for g in range(2):
                    ps_em1 = psB.tile([64, CHUNK], F32, tag="em1", name="ps_em1")
                    ps = ps_em1
                    nc.tensor.matmul(ps, em1w[g][0],
                                     xp[img][g][:, 1 + h0:1 + h0 + CH, 1:1 + W],
                                     start=True, stop=False)
                    nc.tensor.matmul(ps, em1w[g][1],
                                     kt[img][g][:, h0:h0 + CH, :],
                                     start=False, stop=True)
                    nc.scalar.activation(
                        out=uex[img][g][0:64, csl], in_=ps, func=AF.Relu,
                        bias=bem1[g], scale=1.0)
            # D: c1 -> v
            for g in range(2):
                v = vpool.tile([128, HP, WP], BF16, tag="vp")
                pad_guards(v)
                for c in range(NCH):
                    h0 = c * CH
                    ps = psC.tile([128, CHUNK], F32, tag="c1", name="ps")
                    nc.tensor.matmul(ps, c1w[g],
                                     xp[img][g][:, 1 + h0:1 + h0 + CH, 1:1 + W],
                                     start=True, stop=True)
                    nc.scalar.activation(
                        out=v[:, 1 + h0:1 + h0 + CH, 1:1 + W], in_=ps,
                        func=AF.Identity, bias=bv[g], scale=1.0)
                vp[img][g] = v
            # E: em2 pass1 stats
            psg = psD.tile([32, 2], F32, tag="small")
            nmm = 0
            for g in range(2):
                for mt, mp_ in ((0, 128), (1, 16)):
                    st6 = spool.tile([mp_, NCH, 6], F32, tag=f"bnst{mt}")
                    for c in range(NCH):
                        csl = slice(c * CHUNK, (c + 1) * CHUNK)
                        ps_mid = psM.tile([128, CHUNK], F32, tag="em2c", name="ps_mid")
                        ps = ps_mid[0:mp_]
                        nc.tensor.matmul(ps, em2c[g][mt],
                                         uex[img][g][0:64, csl],
                                         start=True, stop=True)
                        nc.vector.bn_stats(out=st6[:, c, :], in_=ps)
                    mv = spool.tile([mp_, 2], F32, tag=f"mv{mt}")
                    nc.vector.bn_aggr(out=mv, in_=st6)
                    rst = spool.tile([mp_, 2], F32, tag=f"rst{mt}")
                    nc.vector.tensor_scalar(out=rst[:, 0:1], in0=mv[:, 0:1],
                                            scalar1=bem2v[g][mt], scalar2=None,
                                            op0=ALU.add)
                    tmp1 = spool.tile([mp_, 1], F32, tag=f"tmp{mt}")
                    nc.scalar.activation(out=tmp1, in_=rst[:, 0:1],
                                         func=AF.Square)
                    nc.vector.tensor_tensor(rst[:, 1:2], mv[:, 1:2], tmp1,
                                            ALU.add)
                    nc.tensor.matmul(psg, ind[g][mt], rst,
                                     start=(nmm == 0), stop=(nmm == 3))
                    nmm += 1
            gst = spool.tile([32, 2], F32, tag="gst")
            nc.scalar.activation(out=gst, in_=psg, func=AF.Identity,
                                 scale=1.0 / 9.0)
            tmp2 = spool.tile([32, 1], F32, tag="tmp2")
            nc.scalar.activation(out=tmp2, in_=gst[:, 0:1], func=AF.Square)
            varv = spool.tile([32, 1], F32, tag="varv")
            nc.vector.tensor_tensor(varv, gst[:, 1:2], tmp2, ALU.subtract)
            nc.scalar.activation(out=varv, in_=varv, func=AF.Sqrt,
                                 bias=epsv[0:32], scale=1.0)
            spt = spool.tile([32, 2], F32, tag="spt")
            nc.vector.reciprocal(out=spt[:, 1:2], in_=varv)
            nc.vector.tensor_copy(out=spt[:, 0:1], in_=gst[:, 0:1])
            pst = psD.tile([2, 32], F32, tag="small")
            nc.tensor.transpose(pst, spt, id32)
            stT = spool.tile([2, 32], F32, tag="stT")
            nc.scalar.activation(out=stT, in_=pst, func=AF.Copy)
            # build scaled lhsT for em2x
            # expand group stats [2,32] -> per-dwg c'-vectors [2,128] in DRAM
            for g in range(2):
                vex = spool.tile([2, 16, 8], BF16, tag="vex", name=f"vex{g}")
                nc.vector.tensor_copy(
                    out=vex,
                    in_=stT[0:2, g * 16:g * 16 + 16][:, :, None].to_broadcast(
                        [2, 16, 8]))
                nc.sync.dma_start(
                    out=bass.AP(tensor=scr2_d, offset=img * 512 + g * 256,
                                ap=[[128, 2], [1, 128]]),
                    in_=vex.rearrange("p a b -> p (a b)"))
            for g in range(2):
                # mu/rho broadcast over the 9 tap rows: [9, 128]
                mu9 = spool.tile([9, 128], BF16, tag="mu9", name=f"mu9{g}")
                nc.sync.dma_start(out=mu9, in_=bass.AP(
                    tensor=scr2_d, offset=img * 512 + g * 256,
                    ap=[[0, 9], [1, 128]]))
                rho9 = spool.tile([9, 128], BF16, tag="rho9", name=f"rho9{g}")
                nc.sync.dma_start(out=rho9, in_=bass.AP(
                    tensor=scr2_d, offset=img * 512 + g * 256 + 128,
                    ap=[[0, 9], [1, 128]]))
                la = lpool.tile([65, 9, 128], BF16, tag="lall")
                lall[img][g] = la
                # beta row: ((be2*gng) - mu*gng) * rho + gnb, in [9,128] layout
                bt = spool.tile([9, 128], F32, tag="bt", name=f"bt{g}")
                nc.vector.tensor_tensor(bt, mu9, gngT[g], ALU.mult)
                nc.vector.tensor_tensor(bt, begT[g], bt, ALU.subtract)
                nc.vector.tensor_tensor(bt, bt, rho9, ALU.mult)
                btb = spool.tile([9, 128], BF16, tag="btb", name=f"btb{g}")
                nc.vector.tensor_tensor(btb, bt, gnbT[g], ALU.add)
                nc.sync.dma_start(
                    out=la[64:65, :, :].rearrange("p a b -> p (a b)"),
                    in_=btb)
                r64 = spool.tile([64, 9, 128], BF16, tag="r64")
                nc.gpsimd.dma_start(out=r64, in_=bass.AP(
                    tensor=scr2_d, offset=img * 512 + g * 256 + 128,
                    ap=[[0, 64], [0, 9], [1, 128]]))
                nc.vector.tensor_tensor(la[0:64], w2x[g], r64, ALU.mult)

        def stage_FG(img):
            for g in range(2):
                ytile = ypool.tile([128, H, W], BF16, tag="y")
                gyt = spool.tile([128, NCH], F32, tag="gapy")
                for c in range(NCH):
                    h0 = c * CH
                    csl = slice(c * CHUNK, (c + 1) * CHUNK)
                    wch = wpool.tile([128, 9, CH, W], BF16, tag="wch")
                    for r in range(3):
                        for kw in range(3):
                            t = r * 3 + kw
                            ps = psE.tile([128, CH, W], F32, tag="em2x")
                            nc.tensor.matmul(
                                ps.rearrange("p a b -> p (a b)"),
                                lall[img][g][:, t, :], uex[img][g][:, csl],
                                start=True, stop=True)
                            nc.scalar.activation(out=wch[:, t], in_=ps,
                                                 func=AF.Identity)
                        # one wide in-place multiply per kh-row against an
                        # overlapping v view: [kw(3) stride 1, a, b]
                        vb = vp[img][g][:, h0 + r:h0 + r + CH, 0:W]
                        vrow = bass.AP(tensor=vb.tensor, offset=vb.offset,
                                       ap=[list(vb.ap[0]), [1, 3], [WP, CH],
                                           [1, W]])
                        wsl = wch[:, 3 * r:3 * r + 3]
                        nc.vector.tensor_tensor(wsl, wsl, vrow, ALU.mult)
                    nc.vector.tensor_tensor(wch[:, 0:4], wch[:, 0:4],
                                            wch[:, 4:8], ALU.add)
                    nc.vector.tensor_tensor(wch[:, 0:2], wch[:, 0:2],
                                            wch[:, 2:4], ALU.add)
                    nc.vector.tensor_tensor(wch[:, 0], wch[:, 0], wch[:, 1],
                                            ALU.add)
                    zc = zpool.tile([128, CH, W], BF16, tag="zc")
                    nc.gpsimd.tensor_tensor(zc, wch[:, 0], wch[:, 8], ALU.add)
                    nc.scalar.activation(
                        out=ytile[:, h0:h0 + CH, :], in_=zc, func=AF.Relu,
                        bias=by[g], scale=1.0, accum_out=gyt[:, c:c + 1])
                yt[img][g] = ytile
                gy[img][g] = gyt

        for img in range(BPC):
            stage_ABCDE(img)
        for img in range(BPC):
            stage_FG(img)

        # ---- SE attention ----
        gapT = [spool.tile([128, BPC], F32, tag=f"gapT{ct}", name=f"gapT{ct}")
                for ct in range(2)]
        for ct in range(2):
            for img in range(BPC):
                rk = spool.tile([128, 1], F32, tag="redk")
                nc.vector.tensor_reduce(out=rk, in_=gk[img][ct],
                                        axis=mybir.AxisListType.X, op=ALU.add)
                ry = spool.tile([128, 1], F32, tag="redy")
                nc.vector.tensor_reduce(out=ry, in_=gy[img][ct],
                                        axis=mybir.AxisListType.X, op=ALU.add)
                nc.vector.tensor_tensor(rk, rk, ry, ALU.add)
                nc.vector.tensor_scalar_mul(gapT[ct][:, img:img + 1], rk,
                                            1.0 / float(NPIX))
        ps_se = psD.tile([128, BPC], F32, tag="small")
        nc.tensor.matmul(ps_se, se1w[0], gapT[0], start=True, stop=False)
        nc.tensor.matmul(ps_se, se1w[1], gapT[1], start=False, stop=True)
        tT = spool.tile([128, BPC], F32, tag="tT")
        nc.scalar.activation(out=tT, in_=ps_se, func=AF.Relu, bias=bse1,
                             scale=1.0)
        a0T = [spool.tile([128, BPC], F32, tag=f"a0T{ct}", name=f"a0T{ct}")
               for ct in range(2)]
        for ct in range(2):
            ps2 = psD.tile([128, BPC], F32, tag="small")
            nc.tensor.matmul(ps2, se2dw[ct], tT, start=True, stop=True)
            nc.scalar.activation(out=a0T[ct], in_=ps2, func=AF.Sigmoid,
                                 bias=db_t[ct], scale=1.0)

        # ---- final mix: out = a0*(y-k) + k ----
        for img in range(BPC):
            for ct in range(2):
                for c in range(NCH):
                    h0 = c * CH
                    dmix = mpool.tile([128, CH, W], BF16, tag="dmix")
                    nc.vector.tensor_tensor(dmix, yt[img][ct][:, h0:h0 + CH],
                                            kt[img][ct][:, h0:h0 + CH],
                                            ALU.subtract)
                    ot = mpool.tile([128, CH, W], BF16, tag="ot")
                    nc.vector.scalar_tensor_tensor(
                        out=ot, in0=dmix, scalar=a0T[ct][:, img:img + 1],
                        in1=kt[img][ct][:, h0:h0 + CH],
                        op0=ALU.mult, op1=ALU.add)
                    nc.sync.dma_start(
                        out=out_d[img, ct * 128:(ct + 1) * 128,
                                  h0:h0 + CH, :],
                        in_=ot)

    _split_waits(nc)
    return nc


# ---------------------------------------------------------------------------
# entry point
# ---------------------------------------------------------------------------
def kernel(**inputs):
    folded = _fold(inputs)
    nc = _build()
    x = inputs["x"].astype(BF)
    in_maps = []
    for core in range(NCORES):
        m = dict(folded)
        m["x_in"] = np.ascontiguousarray(x[core * BPC:(core + 1) * BPC])
        in_maps.append(m)
    res = run_bass_kernel_spmd(nc, in_maps, core_ids=list(range(NCORES)),
                               trace=_trace_flag[0])
    _last_result[0] = res
    out = np.concatenate([np.asarray(r["out"]) for r in res.results], axis=0)
    return out.astype(np.float32)

